# revision 9
# baseline (speedup 1.0000x reference)
"""CPMAnt attention kernel for 8 TRN2 NeuronCores.

Sharding: tensor-parallel over heads. Each core computes 4 of the 32 heads:
  q/k/v projections with column-sliced Wq/Wk/Wv, attention with its slice of
  position_bias, and a partial output projection with the row-sliced Wo.
The 8 partial outputs [B,S,D] (fp16) are summed on the host (the all-reduce).

Device layout trick: the host pre-transposes hidden to hT = hidden^T [D, B*S]
so every matmul on the device uses natural (non-transposed) operand loads:
  qT/kT [dh, rows] = Wx^T-slice @ hidden^T   (lhsT = Wx tiles, rhs = hT tiles)
  v     [rows, dh] = hidden @ Wv-slice       (lhsT = hT tiles, rhs = Wv tiles)
  scores[q, k]     = qh^T.T @ kh^T
  probsT[k, q]     = PE-transpose of softmax(scores)
  ctxT  [dh, q]    = v-tiles.T @ probsT
  outT  [D, rows]  = Wo-slice tiles.T @ ctxT        (partial, summed on host)

The position bias + mask enter as a host-precomputed exp(bias) factor:
exp(s+b) = exp(s)*exp(b).  This replaces the fp32-PSUM bias add (1.2us on
DVE per 128x1024 tile) with one fused fp16 multiply-reduce (tensor_tensor_
reduce, ~0.4us) that also produces the softmax row sums.  Masked positions
have exp(b) = 0 exactly, reproducing the reference's post-softmax mask
zeroing.  No max-subtraction: scores ~ N(0,1) and bias ~ N(0,1) for this
problem, so exp(s) < ~250 and exp(s)*exp(b) < ~45000 stay inside fp16 range.

Scheduling: the attention phase software-pipelines the output projection of
row-block n-1 into the per-head dependency stalls of row-block n with a
4/2/2 m-tile split: [scores qs0/qs1 | 4 outproj m | transp qs0, scores qs2,
transp qs1, scores qs3 | 2 outproj m | transp qs2, transp qs3 | 2 outproj m
| ctx].  The late 2-m chunk covers the softmax-chain latency of qs3 so the
ctx matmuls never wait on the last probsT copy.

Engine balance in attention (per head-block, PE ~12.4us): ACT does exp +
half the outproj drains; DVE does the multiply-reduce, reciprocal, probsT
copies, ctx copy + half the drains; GpSimd (idle otherwise, no PSUM port)
does the softmax normalize (tensor_scalar on SBUF fp16).

DMA queues: only the two hardware DGE queues (sync=SP, scalar=ACT) are
used - the gpsimd software-DGE path transfers at ~25GB/s and steals SDMA
packet slots for its whole duration, which starved the first-block W/h
loads when the first bias tile rode it.  The first ~7us of the NEFF is a
fixed preamble (engine barriers + register loads); first-block W chunks
(scalar) and h groups (sync) are emitted deadline-ordered right after it.
The three projections share one PSUM pool (bufs=2) so a new projection's
first accumulation group lands in the opposite buffer pair from the
previous projection's draining group (no WAR stall at phase switches).
Bias tiles ride sync, two heads (~26us) ahead.  Output drains alternate
ACT/DVE and all output DMAs ride sync; outputs are fp16 partials.
"""

import math

import numpy as np

B, S, D = 2, 1024, 4096
H, DH = 32, 128
NCORES = 8
HPC = H // NCORES  # heads per core = 4
R = B * S  # 2048 rows
KT = D // 128  # 32 contraction tiles for the projections
NB = R // 512  # 4 row blocks
SCALE = 1.0 / math.sqrt(DH)


def _build_core_kernel(repeat: int = 1):
    import concourse.mybir as mybir
    from concourse import bacc
    from concourse.tile import TileContext
    from concourse.masks import make_identity

    f32 = mybir.dt.float32
    fp16 = mybir.dt.float16
    Exp = mybir.ActivationFunctionType.Exp
    Mult = mybir.AluOpType.mult
    Add = mybir.AluOpType.add

    nc = bacc.Bacc("TRN2")

    hqT = nc.declare_dram_parameter("hqT", [D, R], fp16, isOutput=False)
    hkvT = nc.declare_dram_parameter("hkvT", [D, R], fp16, isOutput=False)
    wq = nc.declare_dram_parameter("wq", [D, 512], fp16, isOutput=False)
    wk = nc.declare_dram_parameter("wk", [D, 512], fp16, isOutput=False)
    wv = nc.declare_dram_parameter("wv", [D, 512], fp16, isOutput=False)
    wo = nc.declare_dram_parameter("wo", [512, D], fp16, isOutput=False)
    bias = nc.declare_dram_parameter("bias", [B, HPC, S, S], fp16, isOutput=False)
    outT = nc.declare_dram_parameter("outT", [D, R], fp16, isOutput=True)

    hq3 = hqT.rearrange("(t p) r -> p t r", p=128)  # [128, 32, 2048]
    hkv3 = hkvT.rearrange("(t p) r -> p t r", p=128)
    wq3 = wq.rearrange("(t p) m -> p t m", p=128)  # [128, 32, 512]
    wk3 = wk.rearrange("(t p) m -> p t m", p=128)
    wv3 = wv.rearrange("(t p) m -> p t m", p=128)
    wo3 = wo.rearrange("(t p) m -> p t m", p=128)  # [128, 4, 4096]
    outT3 = outT.rearrange("(m p) r -> p m r", p=128)  # [128, 32, 2048]

    with TileContext(nc) as tc:
      for _rep in range(repeat):
        with (
            tc.tile_pool(name="persist", bufs=1) as pers,
            tc.tile_pool(name="biasp", bufs=3) as bpool,
            tc.tile_pool(name="probsTp", bufs=2) as ptpool,
            tc.tile_pool(name="attn", bufs=3) as apool,
            tc.tile_pool(name="obuf", bufs=4) as opool,
        ):
            wopool = pers
            spool = apool
            # Persistent SBUF tensors
            qT_s = pers.tile([128, HPC, R], fp16)  # 16KB/part
            kT_s = pers.tile([128, HPC, R], fp16)  # 16KB/part
            v_s = pers.tile([128, 16, 512], fp16)  # 16KB/part
            ctxT_s = pers.tile([128, HPC, R], fp16)  # 16KB/part
            ident = pers.tile([128, 128], fp16)
            make_identity(nc, ident)

            # Wo for the output projection: loaded right after the q/k
            # projections on both HWDGE queues.
            wo_s = wopool.tile([128, HPC, D], fp16)  # 32KB/part

            # Bias (= exp(position_bias)*mask, fp16) prefetch: one
            # [128, 4, 1024] tile per (n, h), loaded two heads (~26us)
            # ahead on the sync HWDGE queue -- the 1MB transfer lands as
            # 512 scattered 2KB descriptors, slow enough that a one-head
            # lead left the DVE multiply-reduce racing its arrival.
            def emit_bias_dma(n, h, engine):
                b, qb = divmod(n, 2)
                t = bpool.tile([128, 4, 1024], fp16, tag="bias", name="bias_t")
                engine.dma_start(
                    out=t,
                    in_=bias[b, h].rearrange("(s p) k -> p s k", p=128)[
                        :, qb * 4 : qb * 4 + 4, :
                    ],
                )
                return t

            bias_tiles = {}

            # q/k projections: xT[m, r] += W[kt, m].T @ hT[kt, r]
            # pp: the shared projection PSUM pool (bufs=2)
            def qk_proj(pp, wpool, hpool, w3, hsrc3, dst, scale, first):
                quarters = [
                    wpool.tile([128, 8, 512], fp16, tag="W", name="wh")
                    for _ in range(4)
                ]
                n0_ht = [
                    hpool.tile([128, 4, 512], fp16, tag="ht", name="ht")
                    for _ in range(2)
                ]
                # First-block feed in deadline order: W chunks on scalar,
                # h groups on sync.  kt k is consumed ~0.85us*k after the
                # first matmul; the kt0 operands ride alone (128KB each)
                # so the first matmul starts ~1.2us after the queues open.
                nc.scalar.dma_start(out=quarters[0][:, 0:1, :], in_=w3[:, 0:1, :])
                nc.sync.dma_start(out=n0_ht[0][:, 0:1, :], in_=hsrc3[:, 0:1, 0:512])
                nc.scalar.dma_start(out=quarters[0][:, 1:4, :], in_=w3[:, 1:4, :])
                nc.sync.dma_start(out=n0_ht[0][:, 1:4, :], in_=hsrc3[:, 1:4, 0:512])
                nc.scalar.dma_start(out=quarters[0][:, 4:8, :], in_=w3[:, 4:8, :])
                nc.sync.dma_start(out=n0_ht[1], in_=hsrc3[:, 4:8, 0:512])
                for qt in range(1, 4):
                    nc.scalar.dma_start(
                        out=quarters[qt], in_=w3[:, qt * 8 : (qt + 1) * 8, :]
                    )
                if first:
                    # first bias tiles: sync is otherwise just the h
                    # stream here; lands by ~35us, needed at ~360us.
                    bias_tiles[(0, 0)] = emit_bias_dma(0, 0, nc.sync)
                    bias_tiles[(0, 1)] = emit_bias_dma(0, 1, nc.scalar)
                for n in range(NB):
                    psums = [
                        pp.tile([128, 512], f32, tag=f"pp{m}", name=f"pp{m}")
                        for m in range(4)
                    ]
                    for ktg in range(KT // 4):
                        if n == 0 and ktg < 2:
                            ht = n0_ht[ktg]
                        else:
                            ht = hpool.tile([128, 4, 512], fp16, tag="ht", name="ht")
                            heng = nc.sync if (n == 0 or ktg % 2 == 0) else nc.scalar
                            heng.dma_start(
                                out=ht,
                                in_=hsrc3[:, ktg * 4 : (ktg + 1) * 4, n * 512 : (n + 1) * 512],
                            )
                        for kl in range(4):
                            kt = ktg * 4 + kl
                            wh = quarters[kt // 8]
                            for m in range(4):
                                nc.tensor.matmul(
                                    psums[m],
                                    wh[:, kt % 8, m * 128 : (m + 1) * 128],
                                    ht[:, kl, :],
                                    start=(kt == 0),
                                    stop=(kt == KT - 1),
                                )
                    for m in range(4):
                        nc.scalar.mul(
                            out=dst[:, m, n * 512 : (n + 1) * 512],
                            in_=psums[m],
                            mul=scale,
                        )

            # v projection: v[r, c] += hT[kt, r].T @ Wv[kt, c]
            def v_proj(pp, wpool, hpool):
                quarters = []
                for qt in range(4):
                    wh = wpool.tile([128, 8, 512], fp16, tag="W", name="wh")
                    nc.scalar.dma_start(
                        out=wh, in_=wv3[:, qt * 8 : (qt + 1) * 8, :]
                    )
                    quarters.append(wh)
                for rtg in range(4):  # groups of 4 row-tiles
                    psums = [
                        pp.tile([128, 512], f32, tag=f"pp{j}", name=f"pp{j}")
                        for j in range(4)
                    ]
                    for ktg in range(KT // 4):
                        ht = hpool.tile([128, 4, 512], fp16, tag="ht", name="ht")
                        heng = nc.sync if (rtg == 0 or ktg % 2 == 0) else nc.scalar
                        heng.dma_start(
                            out=ht,
                            in_=hkv3[:, ktg * 4 : (ktg + 1) * 4, rtg * 512 : (rtg + 1) * 512],
                        )
                        for kl in range(4):
                            kt = ktg * 4 + kl
                            wh = quarters[kt // 8]
                            for j in range(4):
                                nc.tensor.matmul(
                                    psums[j],
                                    ht[:, kl, j * 128 : (j + 1) * 128],
                                    wh[:, kt % 8, :],
                                    start=(kt == 0),
                                    stop=(kt == KT - 1),
                                )
                    for j in range(4):
                        nc.scalar.copy(out=v_s[:, rtg * 4 + j, :], in_=psums[j])

            with (
                tc.tile_pool(name="wpool", bufs=4) as wpool,
                tc.tile_pool(name="hstream", bufs=5) as hpool,
                tc.tile_pool(name="ppsum", bufs=2, space="PSUM") as pp,
            ):
                qk_proj(pp, wpool, hpool, wq3, hq3, qT_s, SCALE, first=True)
                qk_proj(pp, wpool, hpool, wk3, hkv3, kT_s, 1.0, first=False)
                v_proj(pp, wpool, hpool)
                bias_tiles[(0, 2)] = emit_bias_dma(0, 2, nc.sync)
                nc.sync.dma_start(out=wo_s[:, 0:2, :], in_=wo3[:, 0:2, :])
                nc.scalar.dma_start(out=wo_s[:, 2:4, :], in_=wo3[:, 2:4, :])

            # attention + output projection, software-pipelined per 512-row
            # block: outproj(n-1) m-tiles fill the softmax stalls of block n.
            with (
                tc.tile_pool(name="spsum", bufs=2, space="PSUM") as sps,
                tc.tile_pool(name="tpsum", bufs=1, space="PSUM") as tps,
                tc.tile_pool(name="cpsum", bufs=1, space="PSUM") as cps,
                tc.tile_pool(name="opsum", bufs=2, space="PSUM") as ops,
            ):
                def scores_softmax(n, h, qs, bias_t):
                    """PE: 2 score MMs.  ACT: exp.  DVE: x exp(bias) fused
                    multiply-reduce + reciprocal.  GpSimd: normalize."""
                    b, qb = divmod(n, 2)
                    q0 = n * 512 + qs * 128  # global row
                    s_ps = sps.tile([128, 1024], f32, tag="s", name="s_ps")
                    for kb in range(2):
                        nc.tensor.matmul(
                            s_ps[:, kb * 512 : (kb + 1) * 512],
                            qT_s[:, h, q0 : q0 + 128],
                            kT_s[
                                :, h, b * 1024 + kb * 512 : b * 1024 + (kb + 1) * 512
                            ],
                            start=True,
                            stop=True,
                        )
                    probsE = apool.tile([128, 1024], fp16, tag="probsE", name="probsE")
                    nc.scalar.activation(out=probsE, in_=s_ps, func=Exp)
                    probsU = apool.tile([128, 1024], fp16, tag="probsU", name="probsU")
                    rowsum = spool.tile([128, 1], f32, tag="rowsum", name="rowsum")
                    nc.vector.scalar_tensor_tensor(
                        out=probsU,
                        in0=probsE,
                        scalar=1.0,
                        in1=bias_t[:, qs, :],
                        op0=Mult,
                        op1=Mult,
                        accum_out=rowsum,
                    )
                    recip = spool.tile([128, 1], f32, tag="recip", name="recip")
                    nc.vector.reciprocal(out=recip, in_=rowsum)
                    # PE transpose_mode ignores the identity operand's VALUES
                    # (pure transpose datapath), so the softmax normalization
                    # must happen before the transpose.  In-place over probsU;
                    # GpSimd has no PSUM port but this is SBUF fp16 - its one
                    # job in the attention phase.
                    nc.gpsimd.tensor_scalar_mul(out=probsU, in0=probsU, scalar1=recip)
                    return probsU

                def transposes(probsN, probsT, qs):
                    """PE: 8 transposes into one PSUM bank, then 1 copy out
                    (DVE)."""
                    t_ps = tps.tile([128, 1024], fp16, tag="t", name="t_ps")
                    for kk in range(8):
                        nc.tensor.transpose(
                            t_ps[:, kk * 128 : (kk + 1) * 128],
                            probsN[:, kk * 128 : (kk + 1) * 128],
                            ident,
                        )
                    nc.vector.tensor_copy(
                        out=probsT[:, :, qs * 128 : (qs + 1) * 128],
                        in_=t_ps.rearrange("p (j q) -> p j q", j=8),
                    )

                def ctx(n, h, probsT):
                    b = n // 2
                    c_ps = cps.tile([128, 512], f32, tag="c", name="c_ps")
                    for kt in range(8):
                        nc.tensor.matmul(
                            c_ps,
                            v_s[:, b * 8 + kt, h * 128 : (h + 1) * 128],
                            probsT[:, kt, :],
                            start=(kt == 0),
                            stop=(kt == 7),
                        )
                    nc.vector.tensor_copy(
                        out=ctxT_s[:, h, n * 512 : (n + 1) * 512], in_=c_ps
                    )

                def outproj_chunk(n, ms, final=False):
                    for m in ms:
                        o_ps = ops.tile([128, 512], f32, tag="o", name="o_ps")
                        for t in range(HPC):
                            nc.tensor.matmul(
                                o_ps,
                                wo_s[:, t, m * 128 : (m + 1) * 128],
                                ctxT_s[:, t, n * 512 : (n + 1) * 512],
                                start=(t == 0),
                                stop=(t == HPC - 1),
                            )
                        osb = opool.tile([128, 512], fp16, tag="osb", name="osb")
                        # drains alternate ACT/DVE; every DMA issues from
                        # sync so neither compute queue eats the 0.6us
                        # issue cost.
                        if m % 2 == 1:
                            nc.vector.tensor_copy(out=osb, in_=o_ps)
                        else:
                            nc.scalar.copy(out=osb, in_=o_ps)
                        nc.sync.dma_start(
                            out=outT3[:, m, n * 512 : (n + 1) * 512], in_=osb
                        )

                heads = [(n, h) for n in range(NB) for h in range(HPC)]
                for idx, (n, h) in enumerate(heads):
                    for ahead in (1, 2):
                        if (
                            idx + ahead < len(heads)
                            and heads[idx + ahead] not in bias_tiles
                        ):
                            bias_tiles[heads[idx + ahead]] = emit_bias_dma(
                                *heads[idx + ahead], nc.sync
                            )
                    bias_t = bias_tiles.pop((n, h))
                    probsT = ptpool.tile(
                        [128, 8, 512], fp16, tag="probsT", name="probsT"
                    )
                    pN = [None] * 4
                    pN[0] = scores_softmax(n, h, 0, bias_t)
                    pN[1] = scores_softmax(n, h, 1, bias_t)
                    if n > 0:
                        outproj_chunk(n - 1, range(h * 8, h * 8 + 4))
                    transposes(pN[0], probsT, 0)
                    pN[2] = scores_softmax(n, h, 2, bias_t)
                    transposes(pN[1], probsT, 1)
                    pN[3] = scores_softmax(n, h, 3, bias_t)
                    if n > 0:
                        outproj_chunk(n - 1, range(h * 8 + 4, h * 8 + 6))
                    transposes(pN[2], probsT, 2)
                    transposes(pN[3], probsT, 3)
                    if n > 0:
                        outproj_chunk(n - 1, range(h * 8 + 6, h * 8 + 8))
                    ctx(n, h, probsT)
                outproj_chunk(NB - 1, range(KT), final=True)

    nc.compile()
    return nc


_NC_CACHE = None


def _prep_in_maps(
    hidden_q, hidden_kv, attention_mask, position_bias, Wq, Wk, Wv, Wo
):
    hqT = np.ascontiguousarray(
        np.asarray(hidden_q, dtype=np.float32).reshape(R, D).T
    ).astype(np.float16)
    hkvT = np.ascontiguousarray(
        np.asarray(hidden_kv, dtype=np.float32).reshape(R, D).T
    ).astype(np.float16)
    mask = np.asarray(attention_mask)
    pb = np.asarray(position_bias, dtype=np.float32)

    in_maps = []
    for c in range(NCORES):
        h0 = c * HPC
        # exp(bias) with masked positions exactly 0: exp(s+b) = exp(s)*exp(b)
        expb_c = np.where(
            mask[:, None, :, :], np.exp(pb[:, h0 : h0 + HPC]), np.float32(0.0)
        )
        in_maps.append(
            {
                "hqT": hqT,
                "hkvT": hkvT,
                "wq": np.ascontiguousarray(Wq[:, h0 * DH : (h0 + HPC) * DH]).astype(np.float16),
                "wk": np.ascontiguousarray(Wk[:, h0 * DH : (h0 + HPC) * DH]).astype(np.float16),
                "wv": np.ascontiguousarray(Wv[:, h0 * DH : (h0 + HPC) * DH]).astype(np.float16),
                "wo": np.ascontiguousarray(
                    Wo[h0 * DH : (h0 + HPC) * DH, :]
                ).astype(np.float16),
                "bias": expb_c.astype(np.float16),
            }
        )
    return in_maps


def kernel(
    hidden_q: np.ndarray,
    hidden_kv: np.ndarray,
    attention_mask: np.ndarray,
    position_bias: np.ndarray,
    Wq: np.ndarray,
    Wk: np.ndarray,
    Wv: np.ndarray,
    Wo: np.ndarray,
) -> np.ndarray:
    from concourse.bass_utils import run_bass_kernel_spmd

    global _NC_CACHE
    if _NC_CACHE is None:
        _NC_CACHE = _build_core_kernel()
    nc = _NC_CACHE

    in_maps = _prep_in_maps(
        hidden_q, hidden_kv, attention_mask, position_bias, Wq, Wk, Wv, Wo
    )
    res = run_bass_kernel_spmd(nc, in_maps, list(range(NCORES)))
    acc = res.results[0]["outT"].astype(np.float32)
    for c in range(1, NCORES):
        acc += res.results[c]["outT"]
    return np.ascontiguousarray(acc.T).reshape(B, S, D)


# revision 10
# speedup vs baseline: 2.2538x; 2.2538x over previous
"""CPMAnt attention kernel for 8 TRN2 NeuronCores.

Sharding: tensor-parallel over heads. Each core computes 4 of the 32 heads:
  q/k/v projections with column-sliced Wq/Wk/Wv, attention with its slice of
  position_bias, and a partial output projection with the row-sliced Wo.
The 8 partial outputs [B,S,D] (fp16) are summed on the host (the all-reduce).

Device layout trick: the host pre-transposes hidden to hT = hidden^T [D, B*S]
so every matmul on the device uses natural (non-transposed) operand loads:
  qT/kT [dh, rows] = Wx^T-slice @ hidden^T   (lhsT = Wx tiles, rhs = hT tiles)
  v     [rows, dh] = hidden @ Wv-slice       (lhsT = hT tiles, rhs = Wv tiles)
  scores[q, k]     = qh^T.T @ kh^T
  probsT[k, q]     = PE-transpose of softmax(scores)
  ctxT  [dh, q]    = v-tiles.T @ probsT
  outT  [D, rows]  = Wo-slice tiles.T @ ctxT        (partial, summed on host)

The position bias + mask enter as a host-precomputed exp(bias) factor:
exp(s+b) = exp(s)*exp(b).  This replaces the fp32-PSUM bias add (1.2us on
DVE per 128x1024 tile) with one fused fp16 multiply-reduce (tensor_tensor_
reduce, ~0.4us) that also produces the softmax row sums.  Masked positions
have exp(b) = 0 exactly, reproducing the reference's post-softmax mask
zeroing.  No max-subtraction: scores ~ N(0,1) and bias ~ N(0,1) for this
problem, so exp(s) < ~250 and exp(s)*exp(b) < ~45000 stay inside fp16 range.

Scheduling: the attention phase software-pipelines the output projection of
row-block n-1 into the per-head dependency stalls of row-block n with a
4/2/2 m-tile split: [scores qs0/qs1 | 4 outproj m | transp qs0, scores qs2,
transp qs1, scores qs3 | 2 outproj m | transp qs2, transp qs3 | 2 outproj m
| ctx].  The late 2-m chunk covers the softmax-chain latency of qs3 so the
ctx matmuls never wait on the last probsT copy.

Engine balance in attention (per head-block, PE ~12.4us): ACT does exp +
half the outproj drains; DVE does the multiply-reduce, reciprocal, probsT
copies, ctx copy + half the drains; GpSimd (idle otherwise, no PSUM port)
does the softmax normalize (tensor_scalar on SBUF fp16).

DMA queues: only the two hardware DGE queues (sync=SP, scalar=ACT) are
used - the gpsimd software-DGE path transfers at ~25GB/s and steals SDMA
packet slots for its whole duration, which starved the first-block W/h
loads when the first bias tile rode it.  The first ~7us of the NEFF is a
fixed preamble (engine barriers + register loads); first-block W chunks
(scalar) and h groups (sync) are emitted deadline-ordered right after it.
The three projections share one PSUM pool (bufs=2) so a new projection's
first accumulation group lands in the opposite buffer pair from the
previous projection's draining group (no WAR stall at phase switches).
Bias tiles ride sync, two heads (~26us) ahead.  Output drains alternate
ACT/DVE and all output DMAs ride sync; outputs are fp16 partials.
"""

import math

import numpy as np

B, S, D = 2, 1024, 4096
H, DH = 32, 128
NCORES = 8
HPC = H // NCORES  # heads per core = 4
R = B * S  # 2048 rows
KT = D // 128  # 32 contraction tiles for the projections
NB = R // 512  # 4 row blocks
SCALE = 1.0 / math.sqrt(DH)


def _build_core_kernel(repeat: int = 1):
    import concourse.mybir as mybir
    from concourse import bacc
    from concourse.tile import TileContext
    from concourse.masks import make_identity

    f32 = mybir.dt.float32
    fp16 = mybir.dt.float16
    Exp = mybir.ActivationFunctionType.Exp
    Mult = mybir.AluOpType.mult
    Add = mybir.AluOpType.add

    nc = bacc.Bacc("TRN2")

    hqT = nc.declare_dram_parameter("hqT", [D, R], fp16, isOutput=False)
    hkvT = nc.declare_dram_parameter("hkvT", [D, R], fp16, isOutput=False)
    wq = nc.declare_dram_parameter("wq", [D, 512], fp16, isOutput=False)
    wk = nc.declare_dram_parameter("wk", [D, 512], fp16, isOutput=False)
    wv = nc.declare_dram_parameter("wv", [D, 512], fp16, isOutput=False)
    wo = nc.declare_dram_parameter("wo", [512, D], fp16, isOutput=False)
    bias = nc.declare_dram_parameter("bias", [B, HPC, S, S], fp16, isOutput=False)
    outT = nc.declare_dram_parameter("outT", [D, R], fp16, isOutput=True)

    hq3 = hqT.rearrange("(t p) r -> p t r", p=128)  # [128, 32, 2048]
    hkv3 = hkvT.rearrange("(t p) r -> p t r", p=128)
    wq3 = wq.rearrange("(t p) m -> p t m", p=128)  # [128, 32, 512]
    wk3 = wk.rearrange("(t p) m -> p t m", p=128)
    wv3 = wv.rearrange("(t p) m -> p t m", p=128)
    wo3 = wo.rearrange("(t p) m -> p t m", p=128)  # [128, 4, 4096]
    outT3 = outT.rearrange("(m p) r -> p m r", p=128)  # [128, 32, 2048]

    with TileContext(nc) as tc:
      for _rep in range(repeat):
        with (
            tc.tile_pool(name="persist", bufs=1) as pers,
            tc.tile_pool(name="biasp", bufs=3) as bpool,
            tc.tile_pool(name="probsTp", bufs=2) as ptpool,
            tc.tile_pool(name="attn", bufs=3) as apool,
            tc.tile_pool(name="obuf", bufs=4) as opool,
        ):
            wopool = pers
            spool = apool
            # Persistent SBUF tensors
            qT_s = pers.tile([128, HPC, R], fp16)  # 16KB/part
            kT_s = pers.tile([128, HPC, R], fp16)  # 16KB/part
            v_s = pers.tile([128, 16, 512], fp16)  # 16KB/part
            ctxT_s = pers.tile([128, HPC, R], fp16)  # 16KB/part
            ident = pers.tile([128, 128], fp16)
            make_identity(nc, ident)

            # Wo for the output projection: loaded right after the q/k
            # projections on both HWDGE queues.
            wo_s = wopool.tile([128, HPC, D], fp16)  # 32KB/part

            # Bias (= exp(position_bias)*mask, fp16) prefetch: one
            # [128, 4, 1024] tile per (n, h), loaded two heads (~26us)
            # ahead on the sync HWDGE queue -- the 1MB transfer lands as
            # 512 scattered 2KB descriptors, slow enough that a one-head
            # lead left the DVE multiply-reduce racing its arrival.
            def emit_bias_dma(n, h, engine):
                b, qb = divmod(n, 2)
                t = bpool.tile([128, 4, 1024], fp16, tag="bias", name="bias_t")
                engine.dma_start(
                    out=t,
                    in_=bias[b, h].rearrange("(s p) k -> p s k", p=128)[
                        :, qb * 4 : qb * 4 + 4, :
                    ],
                )
                return t

            bias_tiles = {}

            # q/k projections: xT[m, r] += W[kt, m].T @ hT[kt, r]
            # pp: the shared projection PSUM pool (bufs=2)
            def qk_proj(pp, wpool, hpool, w3, hsrc3, dst, scale, first):
                quarters = [
                    wpool.tile([128, 8, 512], fp16, tag="W", name="wh")
                    for _ in range(4)
                ]
                n0_ht = [
                    hpool.tile([128, 4, 512], fp16, tag="ht", name="ht")
                    for _ in range(2)
                ]
                # First-block feed in deadline order: W chunks on scalar,
                # h groups on sync.  kt k is consumed ~0.85us*k after the
                # first matmul; the kt0 operands ride alone (128KB each)
                # so the first matmul starts ~1.2us after the queues open.
                nc.scalar.dma_start(out=quarters[0][:, 0:1, :], in_=w3[:, 0:1, :])
                nc.sync.dma_start(out=n0_ht[0][:, 0:1, :], in_=hsrc3[:, 0:1, 0:512])
                nc.scalar.dma_start(out=quarters[0][:, 1:4, :], in_=w3[:, 1:4, :])
                nc.sync.dma_start(out=n0_ht[0][:, 1:4, :], in_=hsrc3[:, 1:4, 0:512])
                nc.scalar.dma_start(out=quarters[0][:, 4:8, :], in_=w3[:, 4:8, :])
                nc.sync.dma_start(out=n0_ht[1], in_=hsrc3[:, 4:8, 0:512])
                for qt in range(1, 4):
                    nc.scalar.dma_start(
                        out=quarters[qt], in_=w3[:, qt * 8 : (qt + 1) * 8, :]
                    )
                if first:
                    # first bias tiles: sync is otherwise just the h
                    # stream here; lands by ~35us, needed at ~360us.
                    bias_tiles[(0, 0)] = emit_bias_dma(0, 0, nc.sync)
                    bias_tiles[(0, 1)] = emit_bias_dma(0, 1, nc.scalar)
                for n in range(NB):
                    psums = [
                        pp.tile([128, 512], f32, tag=f"pp{m}", name=f"pp{m}")
                        for m in range(4)
                    ]
                    for ktg in range(KT // 4):
                        if n == 0 and ktg < 2:
                            ht = n0_ht[ktg]
                        else:
                            ht = hpool.tile([128, 4, 512], fp16, tag="ht", name="ht")
                            heng = nc.sync if (n == 0 or ktg % 2 == 0) else nc.scalar
                            heng.dma_start(
                                out=ht,
                                in_=hsrc3[:, ktg * 4 : (ktg + 1) * 4, n * 512 : (n + 1) * 512],
                            )
                        for kl in range(4):
                            kt = ktg * 4 + kl
                            wh = quarters[kt // 8]
                            for m in range(4):
                                nc.tensor.matmul(
                                    psums[m],
                                    wh[:, kt % 8, m * 128 : (m + 1) * 128],
                                    ht[:, kl, :],
                                    start=(kt == 0),
                                    stop=(kt == KT - 1),
                                )
                    for m in range(4):
                        nc.scalar.mul(
                            out=dst[:, m, n * 512 : (n + 1) * 512],
                            in_=psums[m],
                            mul=scale,
                        )

            # v projection: v[r, c] += hT[kt, r].T @ Wv[kt, c]
            def v_proj(pp, wpool, hpool):
                quarters = []
                for qt in range(4):
                    wh = wpool.tile([128, 8, 512], fp16, tag="W", name="wh")
                    nc.scalar.dma_start(
                        out=wh, in_=wv3[:, qt * 8 : (qt + 1) * 8, :]
                    )
                    quarters.append(wh)
                for rtg in range(4):  # groups of 4 row-tiles
                    psums = [
                        pp.tile([128, 512], f32, tag=f"pp{j}", name=f"pp{j}")
                        for j in range(4)
                    ]
                    for ktg in range(KT // 4):
                        ht = hpool.tile([128, 4, 512], fp16, tag="ht", name="ht")
                        heng = nc.sync if (rtg == 0 or ktg % 2 == 0) else nc.scalar
                        heng.dma_start(
                            out=ht,
                            in_=hkv3[:, ktg * 4 : (ktg + 1) * 4, rtg * 512 : (rtg + 1) * 512],
                        )
                        for kl in range(4):
                            kt = ktg * 4 + kl
                            wh = quarters[kt // 8]
                            for j in range(4):
                                nc.tensor.matmul(
                                    psums[j],
                                    ht[:, kl, j * 128 : (j + 1) * 128],
                                    wh[:, kt % 8, :],
                                    start=(kt == 0),
                                    stop=(kt == KT - 1),
                                )
                    for j in range(4):
                        nc.scalar.copy(out=v_s[:, rtg * 4 + j, :], in_=psums[j])

            with (
                tc.tile_pool(name="wpool", bufs=4) as wpool,
                tc.tile_pool(name="hstream", bufs=5) as hpool,
                tc.tile_pool(name="ppsum", bufs=2, space="PSUM") as pp,
            ):
                qk_proj(pp, wpool, hpool, wq3, hq3, qT_s, SCALE, first=True)
                qk_proj(pp, wpool, hpool, wk3, hkv3, kT_s, 1.0, first=False)
                v_proj(pp, wpool, hpool)
                bias_tiles[(0, 2)] = emit_bias_dma(0, 2, nc.sync)
                nc.sync.dma_start(out=wo_s[:, 0:2, :], in_=wo3[:, 0:2, :])
                nc.scalar.dma_start(out=wo_s[:, 2:4, :], in_=wo3[:, 2:4, :])

            # attention + output projection, software-pipelined per 512-row
            # block: outproj(n-1) m-tiles fill the softmax stalls of block n.
            with (
                tc.tile_pool(name="spsum", bufs=2, space="PSUM") as sps,
                tc.tile_pool(name="tpsum", bufs=1, space="PSUM") as tps,
                tc.tile_pool(name="cpsum", bufs=1, space="PSUM") as cps,
                tc.tile_pool(name="opsum", bufs=2, space="PSUM") as ops,
            ):
                def scores_softmax(n, h, qs, bias_t):
                    """PE: 2 score MMs.  ACT: exp.  DVE: x exp(bias) fused
                    multiply-reduce + reciprocal.  GpSimd: normalize."""
                    b, qb = divmod(n, 2)
                    q0 = n * 512 + qs * 128  # global row
                    s_ps = sps.tile([128, 1024], f32, tag="s", name="s_ps")
                    for kb in range(2):
                        nc.tensor.matmul(
                            s_ps[:, kb * 512 : (kb + 1) * 512],
                            qT_s[:, h, q0 : q0 + 128],
                            kT_s[
                                :, h, b * 1024 + kb * 512 : b * 1024 + (kb + 1) * 512
                            ],
                            start=True,
                            stop=True,
                        )
                    probsE = apool.tile([128, 1024], fp16, tag="probsE", name="probsE")
                    nc.scalar.activation(out=probsE, in_=s_ps, func=Exp)
                    probsU = apool.tile([128, 1024], fp16, tag="probsU", name="probsU")
                    rowsum = spool.tile([128, 1], f32, tag="rowsum", name="rowsum")
                    nc.vector.tensor_mul(out=probsU, in0=probsE, in1=bias_t[:, qs, :])
                    nc.vector.tensor_reduce(
                        rowsum, probsU, mybir.AxisListType.X, Add
                    )
                    recip = spool.tile([128, 1], f32, tag="recip", name="recip")
                    nc.vector.reciprocal(out=recip, in_=rowsum)
                    # PE transpose_mode ignores the identity operand's VALUES
                    # (pure transpose datapath), so the softmax normalization
                    # must happen before the transpose.  In-place over probsU;
                    # GpSimd has no PSUM port but this is SBUF fp16 - its one
                    # job in the attention phase.
                    nc.vector.tensor_scalar_mul(out=probsU, in0=probsU, scalar1=recip)
                    return probsU

                def transposes(probsN, probsT, qs):
                    """PE: 8 transposes into one PSUM bank, then 1 copy out
                    (DVE)."""
                    t_ps = tps.tile([128, 1024], fp16, tag="t", name="t_ps")
                    for kk in range(8):
                        nc.tensor.transpose(
                            t_ps[:, kk * 128 : (kk + 1) * 128],
                            probsN[:, kk * 128 : (kk + 1) * 128],
                            ident,
                        )
                    nc.vector.tensor_copy(
                        out=probsT[:, :, qs * 128 : (qs + 1) * 128],
                        in_=t_ps.rearrange("p (j q) -> p j q", j=8),
                    )

                def ctx(n, h, probsT):
                    b = n // 2
                    c_ps = cps.tile([128, 512], f32, tag="c", name="c_ps")
                    for kt in range(8):
                        nc.tensor.matmul(
                            c_ps,
                            v_s[:, b * 8 + kt, h * 128 : (h + 1) * 128],
                            probsT[:, kt, :],
                            start=(kt == 0),
                            stop=(kt == 7),
                        )
                    nc.vector.tensor_copy(
                        out=ctxT_s[:, h, n * 512 : (n + 1) * 512], in_=c_ps
                    )

                def outproj_chunk(n, ms, final=False):
                    for m in ms:
                        o_ps = ops.tile([128, 512], f32, tag="o", name="o_ps")
                        for t in range(HPC):
                            nc.tensor.matmul(
                                o_ps,
                                wo_s[:, t, m * 128 : (m + 1) * 128],
                                ctxT_s[:, t, n * 512 : (n + 1) * 512],
                                start=(t == 0),
                                stop=(t == HPC - 1),
                            )
                        osb = opool.tile([128, 512], fp16, tag="osb", name="osb")
                        # drains alternate ACT/DVE; every DMA issues from
                        # sync so neither compute queue eats the 0.6us
                        # issue cost.
                        if m % 2 == 1:
                            nc.vector.tensor_copy(out=osb, in_=o_ps)
                        else:
                            nc.scalar.copy(out=osb, in_=o_ps)
                        nc.sync.dma_start(
                            out=outT3[:, m, n * 512 : (n + 1) * 512], in_=osb
                        )

                heads = [(n, h) for n in range(NB) for h in range(HPC)]
                for idx, (n, h) in enumerate(heads):
                    for ahead in (1, 2):
                        if (
                            idx + ahead < len(heads)
                            and heads[idx + ahead] not in bias_tiles
                        ):
                            bias_tiles[heads[idx + ahead]] = emit_bias_dma(
                                *heads[idx + ahead], nc.sync
                            )
                    bias_t = bias_tiles.pop((n, h))
                    probsT = ptpool.tile(
                        [128, 8, 512], fp16, tag="probsT", name="probsT"
                    )
                    pN = [None] * 4
                    pN[0] = scores_softmax(n, h, 0, bias_t)
                    pN[1] = scores_softmax(n, h, 1, bias_t)
                    if n > 0:
                        outproj_chunk(n - 1, range(h * 8, h * 8 + 4))
                    transposes(pN[0], probsT, 0)
                    pN[2] = scores_softmax(n, h, 2, bias_t)
                    transposes(pN[1], probsT, 1)
                    pN[3] = scores_softmax(n, h, 3, bias_t)
                    if n > 0:
                        outproj_chunk(n - 1, range(h * 8 + 4, h * 8 + 6))
                    transposes(pN[2], probsT, 2)
                    transposes(pN[3], probsT, 3)
                    if n > 0:
                        outproj_chunk(n - 1, range(h * 8 + 6, h * 8 + 8))
                    ctx(n, h, probsT)
                outproj_chunk(NB - 1, range(KT), final=True)

    nc.compile()
    return nc


_NC_CACHE = None


def _prep_in_maps(
    hidden_q, hidden_kv, attention_mask, position_bias, Wq, Wk, Wv, Wo
):
    hqT = np.ascontiguousarray(
        np.asarray(hidden_q, dtype=np.float32).reshape(R, D).T
    ).astype(np.float16)
    hkvT = np.ascontiguousarray(
        np.asarray(hidden_kv, dtype=np.float32).reshape(R, D).T
    ).astype(np.float16)
    mask = np.asarray(attention_mask)
    pb = np.asarray(position_bias, dtype=np.float32)

    in_maps = []
    for c in range(NCORES):
        h0 = c * HPC
        # exp(bias) with masked positions exactly 0: exp(s+b) = exp(s)*exp(b)
        expb_c = np.where(
            mask[:, None, :, :], np.exp(pb[:, h0 : h0 + HPC]), np.float32(0.0)
        )
        in_maps.append(
            {
                "hqT": hqT,
                "hkvT": hkvT,
                "wq": np.ascontiguousarray(Wq[:, h0 * DH : (h0 + HPC) * DH]).astype(np.float16),
                "wk": np.ascontiguousarray(Wk[:, h0 * DH : (h0 + HPC) * DH]).astype(np.float16),
                "wv": np.ascontiguousarray(Wv[:, h0 * DH : (h0 + HPC) * DH]).astype(np.float16),
                "wo": np.ascontiguousarray(
                    Wo[h0 * DH : (h0 + HPC) * DH, :]
                ).astype(np.float16),
                "bias": expb_c.astype(np.float16),
            }
        )
    return in_maps


def kernel(
    hidden_q: np.ndarray,
    hidden_kv: np.ndarray,
    attention_mask: np.ndarray,
    position_bias: np.ndarray,
    Wq: np.ndarray,
    Wk: np.ndarray,
    Wv: np.ndarray,
    Wo: np.ndarray,
) -> np.ndarray:
    from concourse.bass_utils import run_bass_kernel_spmd

    global _NC_CACHE
    if _NC_CACHE is None:
        _NC_CACHE = _build_core_kernel()
    nc = _NC_CACHE

    in_maps = _prep_in_maps(
        hidden_q, hidden_kv, attention_mask, position_bias, Wq, Wk, Wv, Wo
    )
    res = run_bass_kernel_spmd(nc, in_maps, list(range(NCORES)))
    acc = res.results[0]["outT"].astype(np.float32)
    for c in range(1, NCORES):
        acc += res.results[c]["outT"]
    return np.ascontiguousarray(acc.T).reshape(B, S, D)


# revision 12
# speedup vs baseline: 2.4309x; 1.0786x over previous
"""CPMAnt attention kernel for 8 TRN2 NeuronCores.

Sharding: tensor-parallel over heads. Each core computes 4 of the 32 heads:
  q/k/v projections with column-sliced Wq/Wk/Wv, attention with its slice of
  position_bias, and a partial output projection with the row-sliced Wo.
The 8 partial outputs [B,S,D] (fp16) are summed on the host (the all-reduce).

Device layout trick: the host pre-transposes hidden to hT = hidden^T [D, B*S]
so every matmul on the device uses natural (non-transposed) operand loads:
  qT/kT [dh, rows] = Wx^T-slice @ hidden^T   (lhsT = Wx tiles, rhs = hT tiles)
  v     [rows, dh] = hidden @ Wv-slice       (lhsT = hT tiles, rhs = Wv tiles)
  scores[q, k]     = qh^T.T @ kh^T
  probsT[k, q]     = PE-transpose of softmax(scores)
  ctxT  [dh, q]    = v-tiles.T @ probsT
  outT  [D, rows]  = Wo-slice tiles.T @ ctxT        (partial, summed on host)

softmax: scores+bias are summed on DVE (tensor_add, PSUM fp32 + SBUF fp16
bias -> SBUF fp16), then one ACT exp with accum_out gives probs and row
sums together.  The fp16 rounding of (s+b) (quantum 2^-8 at |x|~6) costs
~4e-3 relative on the largest probs - well inside the 2e-2 budget - and
keeps the softmax chain at ~2.6us (add 0.9 + exp 1.15 + recip 0.15 + norm
0.47); the fused DVE multiply-reduce alternatives measure 9us (SCALAR_
TENSOR_TENSOR microcode) or crash (TENSOR_TENSOR_REDUCE), and a separate
TENSOR_REDUCE is 1.2us.  Masked positions are -30000 so exp gives exactly
0, reproducing the reference's post-softmax mask zeroing.  No max-
subtraction: |s+b| < ~8 for this problem's N(0,1) data.

Scheduling: the v projection is interleaved into block-0 attention - the
32 (rtg, ktg) v work units (16 matmuls each, ~3.4us) fill the softmax-
chain dependency stalls of the first 4 head-blocks, which have no output
projection to pipeline yet (block 0: 11 units up front, then 5 per head,
sized so v rows 0-1023 complete exactly when ctx(0,0) needs them).
Blocks 1-3 pipeline outproj(n-1) m-tiles into the stalls with a 4/2/2
split: [ss0 ss1 | 4m | tr0 ss2 tr1 ss3 | 2m | tr2 tr3 | 2m | ctx]; the
late 2m chunk covers the qs3 softmax chain so ctx never waits.

PSUM: the three projections share one pool (bufs=2, 8 banks) so a new
projection's first accumulation group lands in the opposite buffer pair
(no WAR stall at phase switches).  During the v/attention overlap:
v 4 banks + scores 2 (bufs=1, the v fills cover the serialization) +
transpose 1 + ctx 1 = 8.  Blocks 1-3: scores 4 + transpose 1 + ctx 1 +
outproj 2 = 8.

DMA queues: only the two hardware DGE queues (sync=SP, scalar=ACT) are
used - the gpsimd software-DGE path transfers at ~25GB/s and steals SDMA
packet slots for its whole duration, which starved the first-block W/h
loads when the first bias tile rode it.  The first ~7us of the NEFF is a
fixed preamble; the first W chunk leads the sync queue (the scalar queue
opens ~1.3us later with the ACT exp-table load) and first-block operands
are emitted deadline-ordered, kt0's alone for the earliest first matmul.
Bias tiles ride sync two heads (~26us) ahead.  Output drains alternate
ACT/DVE; outputs are fp16 partials.
"""

import math

import numpy as np

B, S, D = 2, 1024, 4096
H, DH = 32, 128
NCORES = 8
HPC = H // NCORES  # heads per core = 4
R = B * S  # 2048 rows
KT = D // 128  # 32 contraction tiles for the projections
NB = R // 512  # 4 row blocks
SCALE = 1.0 / math.sqrt(DH)
MASK_NEG = -30000.0


def _build_core_kernel(repeat: int = 1):
    import concourse.mybir as mybir
    from concourse import bacc
    from concourse.tile import TileContext
    from concourse.masks import make_identity

    f32 = mybir.dt.float32
    fp16 = mybir.dt.float16
    Exp = mybir.ActivationFunctionType.Exp

    nc = bacc.Bacc("TRN2")

    hqT = nc.declare_dram_parameter("hqT", [D, R], fp16, isOutput=False)
    hkvT = nc.declare_dram_parameter("hkvT", [D, R], fp16, isOutput=False)
    wq = nc.declare_dram_parameter("wq", [D, 512], fp16, isOutput=False)
    wk = nc.declare_dram_parameter("wk", [D, 512], fp16, isOutput=False)
    wv = nc.declare_dram_parameter("wv", [D, 512], fp16, isOutput=False)
    wo = nc.declare_dram_parameter("wo", [512, D], fp16, isOutput=False)
    bias = nc.declare_dram_parameter("bias", [B, HPC, S, S], fp16, isOutput=False)
    outT = nc.declare_dram_parameter("outT", [D, R], fp16, isOutput=True)

    hq3 = hqT.rearrange("(t p) r -> p t r", p=128)  # [128, 32, 2048]
    hkv3 = hkvT.rearrange("(t p) r -> p t r", p=128)
    wq3 = wq.rearrange("(t p) m -> p t m", p=128)  # [128, 32, 512]
    wk3 = wk.rearrange("(t p) m -> p t m", p=128)
    wv3 = wv.rearrange("(t p) m -> p t m", p=128)
    wo3 = wo.rearrange("(t p) m -> p t m", p=128)  # [128, 4, 4096]
    outT3 = outT.rearrange("(m p) r -> p m r", p=128)  # [128, 32, 2048]

    with TileContext(nc) as tc:
      for _rep in range(repeat):
        with (
            tc.tile_pool(name="persist", bufs=1) as pers,
            tc.tile_pool(name="biasp", bufs=3) as bpool,
            tc.tile_pool(name="probsTp", bufs=2) as ptpool,
            tc.tile_pool(name="attn", bufs=3) as apool,
            tc.tile_pool(name="obuf", bufs=4) as opool,
        ):
            wopool = pers
            spool = apool
            # Persistent SBUF tensors
            qT_s = pers.tile([128, HPC, R], fp16)  # 16KB/part
            kT_s = pers.tile([128, HPC, R], fp16)  # 16KB/part
            v_s = pers.tile([128, 16, 512], fp16)  # 16KB/part
            ctxT_s = pers.tile([128, HPC, R], fp16)  # 16KB/part
            ident = pers.tile([128, 128], fp16)
            make_identity(nc, ident)

            # Wo for the output projection: loaded right after the
            # projections on both HWDGE queues.
            wo_s = wopool.tile([128, HPC, D], fp16)  # 32KB/part

            # Bias prefetch: one [128, 4, 1024] tile per (n, h), loaded two
            # heads (~26us) ahead on the sync HWDGE queue -- the 1MB transfer
            # lands as 512 scattered 2KB descriptors, slow enough that a
            # one-head lead left the DVE bias-add racing its arrival.
            def emit_bias_dma(n, h, engine):
                b, qb = divmod(n, 2)
                t = bpool.tile([128, 4, 1024], fp16, tag="bias", name="bias_t")
                engine.dma_start(
                    out=t,
                    in_=bias[b, h].rearrange("(s p) k -> p s k", p=128)[
                        :, qb * 4 : qb * 4 + 4, :
                    ],
                )
                return t

            bias_tiles = {}
            heads = [(n, h) for n in range(NB) for h in range(HPC)]

            def ensure_bias_ahead(idx):
                for ahead in (1, 2):
                    if (
                        idx + ahead < len(heads)
                        and heads[idx + ahead] not in bias_tiles
                    ):
                        bias_tiles[heads[idx + ahead]] = emit_bias_dma(
                            *heads[idx + ahead], nc.sync
                        )

            # q/k projections: xT[m, r] += W[kt, m].T @ hT[kt, r]
            # pp: the shared projection PSUM pool (bufs=2)
            def qk_proj(pp, wpool, hpool, w3, hsrc3, dst, scale, first):
                quarters = [
                    wpool.tile([128, 8, 512], fp16, tag="W", name="wh")
                    for _ in range(4)
                ]
                n0_ht = [
                    hpool.tile([128, 4, 512], fp16, tag="ht", name="ht")
                    for _ in range(2)
                ]
                # First-block feed in deadline order; kt k is consumed
                # ~0.85us*k after the first matmul.  kt0's W leads the sync
                # queue (scalar opens ~1.3us later behind the ACT exp-table
                # load), kt0's h leads scalar; each 128KB.
                nc.sync.dma_start(out=quarters[0][:, 0:1, :], in_=w3[:, 0:1, :])
                nc.scalar.dma_start(
                    out=n0_ht[0][:, 0:1, :], in_=hsrc3[:, 0:1, 0:512]
                )
                nc.sync.dma_start(out=n0_ht[0][:, 1:4, :], in_=hsrc3[:, 1:4, 0:512])
                nc.scalar.dma_start(out=quarters[0][:, 1:4, :], in_=w3[:, 1:4, :])
                nc.sync.dma_start(out=quarters[0][:, 4:8, :], in_=w3[:, 4:8, :])
                nc.scalar.dma_start(out=n0_ht[1], in_=hsrc3[:, 4:8, 0:512])
                for qt in range(1, 4):
                    nc.scalar.dma_start(
                        out=quarters[qt], in_=w3[:, qt * 8 : (qt + 1) * 8, :]
                    )
                if first:
                    # first bias tiles ride the projection-phase queues;
                    # they land by ~35us, needed at ~250us.
                    bias_tiles[(0, 0)] = emit_bias_dma(0, 0, nc.sync)
                    bias_tiles[(0, 1)] = emit_bias_dma(0, 1, nc.scalar)
                for n in range(NB):
                    psums = [
                        pp.tile([128, 512], f32, tag=f"pp{m}", name=f"pp{m}")
                        for m in range(4)
                    ]
                    for ktg in range(KT // 4):
                        if n == 0 and ktg < 2:
                            ht = n0_ht[ktg]
                        else:
                            ht = hpool.tile([128, 4, 512], fp16, tag="ht", name="ht")
                            heng = nc.sync if (n == 0 or ktg % 2 == 0) else nc.scalar
                            heng.dma_start(
                                out=ht,
                                in_=hsrc3[:, ktg * 4 : (ktg + 1) * 4, n * 512 : (n + 1) * 512],
                            )
                        for kl in range(4):
                            kt = ktg * 4 + kl
                            wh = quarters[kt // 8]
                            for m in range(4):
                                nc.tensor.matmul(
                                    psums[m],
                                    wh[:, kt % 8, m * 128 : (m + 1) * 128],
                                    ht[:, kl, :],
                                    start=(kt == 0),
                                    stop=(kt == KT - 1),
                                )
                    for m in range(4):
                        nc.scalar.mul(
                            out=dst[:, m, n * 512 : (n + 1) * 512],
                            in_=psums[m],
                            mul=scale,
                        )

            with (
                tc.tile_pool(name="wpool", bufs=4) as wpool,
                tc.tile_pool(name="hstream", bufs=6) as hpool,
            ):
                with tc.tile_pool(name="ppsum", bufs=2, space="PSUM") as pp:
                    qk_proj(pp, wpool, hpool, wq3, hq3, qT_s, SCALE, first=True)
                    qk_proj(pp, wpool, hpool, wk3, hkv3, kT_s, 1.0, first=False)
                bias_tiles[(0, 2)] = emit_bias_dma(0, 2, nc.sync)
                nc.sync.dma_start(out=wo_s[:, 0:2, :], in_=wo3[:, 0:2, :])
                nc.scalar.dma_start(out=wo_s[:, 2:4, :], in_=wo3[:, 2:4, :])

                # ---- attention (+ v projection overlapped into block 0) ----
                with (
                    tc.tile_pool(name="tpsum", bufs=1, space="PSUM") as tps,
                    tc.tile_pool(name="cpsum", bufs=1, space="PSUM") as cps,
                ):
                    def scores_softmax(sps, n, h, qs, bias_t):
                        """PE: 2 score MMs.  DVE: add bias (PSUM fp32 + SBUF
                        fp16 -> SBUF fp16).  ACT: exp with free row-sum
                        accum.  DVE: reciprocal + normalize."""
                        b, qb = divmod(n, 2)
                        q0 = n * 512 + qs * 128  # global row
                        s_ps = sps.tile([128, 1024], f32, tag="s", name="s_ps")
                        for kb in range(2):
                            nc.tensor.matmul(
                                s_ps[:, kb * 512 : (kb + 1) * 512],
                                qT_s[:, h, q0 : q0 + 128],
                                kT_s[
                                    :,
                                    h,
                                    b * 1024 + kb * 512 : b * 1024 + (kb + 1) * 512,
                                ],
                                start=True,
                                stop=True,
                            )
                        sb = apool.tile(
                            [128, 1024], fp16, tag="sb", name="sb", bufs=2
                        )
                        nc.vector.tensor_add(out=sb, in0=s_ps, in1=bias_t[:, qs, :])
                        probsU = apool.tile(
                            [128, 1024], fp16, tag="probsU", name="probsU"
                        )
                        rowsum = spool.tile([128, 1], f32, tag="rowsum", name="rowsum")
                        nc.scalar.activation(
                            out=probsU, in_=sb, func=Exp, accum_out=rowsum
                        )
                        recip = spool.tile([128, 1], f32, tag="recip", name="recip")
                        nc.vector.reciprocal(out=recip, in_=rowsum)
                        # PE transpose_mode ignores the identity operand's
                        # VALUES (pure transpose datapath), so the softmax
                        # normalization must happen before the transpose.
                        nc.vector.tensor_scalar_mul(
                            out=probsU, in0=probsU, scalar1=recip
                        )
                        return probsU

                    def transposes(probsN, probsT, qs):
                        """PE: 8 transposes into one PSUM bank, then 1 copy
                        out (alternating DVE/ACT to balance engine load)."""
                        t_ps = tps.tile([128, 1024], fp16, tag="t", name="t_ps")
                        for kk in range(8):
                            nc.tensor.transpose(
                                t_ps[:, kk * 128 : (kk + 1) * 128],
                                probsN[:, kk * 128 : (kk + 1) * 128],
                                ident,
                            )
                        dst = probsT[:, :, qs * 128 : (qs + 1) * 128]
                        src = t_ps.rearrange("p (j q) -> p j q", j=8)
                        if qs % 2 == 0:
                            nc.vector.tensor_copy(out=dst, in_=src)
                        else:
                            nc.scalar.copy(out=dst, in_=src)

                    def ctx(n, h, probsT):
                        b = n // 2
                        c_ps = cps.tile([128, 512], f32, tag="c", name="c_ps")
                        for kt in range(8):
                            nc.tensor.matmul(
                                c_ps,
                                v_s[:, b * 8 + kt, h * 128 : (h + 1) * 128],
                                probsT[:, kt, :],
                                start=(kt == 0),
                                stop=(kt == 7),
                            )
                        nc.vector.tensor_copy(
                            out=ctxT_s[:, h, n * 512 : (n + 1) * 512], in_=c_ps
                        )

                    def outproj_chunk(ops, n, ms):
                        for m in ms:
                            o_ps = ops.tile([128, 512], f32, tag="o", name="o_ps")
                            for t in range(HPC):
                                nc.tensor.matmul(
                                    o_ps,
                                    wo_s[:, t, m * 128 : (m + 1) * 128],
                                    ctxT_s[:, t, n * 512 : (n + 1) * 512],
                                    start=(t == 0),
                                    stop=(t == HPC - 1),
                                )
                            osb = opool.tile([128, 512], fp16, tag="osb", name="osb")
                            # drains alternate ACT (3 of 8) / DVE (5 of 8);
                            # every DMA issues from sync.
                            if m % 8 < 3:
                                nc.scalar.copy(out=osb, in_=o_ps)
                            else:
                                nc.vector.tensor_copy(out=osb, in_=o_ps)
                            nc.sync.dma_start(
                                out=outT3[:, m, n * 512 : (n + 1) * 512], in_=osb
                            )

                    # --- v projection work units: (rtg, ktg), 16 MMs each ---
                    with (
                        tc.tile_pool(name="vpsum", bufs=1, space="PSUM") as vp,
                        tc.tile_pool(name="spsum0", bufs=1, space="PSUM") as sps0,
                    ):
                        vq = [
                            wpool.tile([128, 8, 512], fp16, tag="W", name="wh")
                            for _ in range(4)
                        ]
                        for qt in range(4):
                            nc.scalar.dma_start(
                                out=vq[qt], in_=wv3[:, qt * 8 : (qt + 1) * 8, :]
                            )
                        vpsums = {}

                        def v_unit(rtg, ktg):
                            if ktg == 0:
                                vpsums[rtg] = [
                                    vp.tile(
                                        [128, 512], f32, tag=f"vp{j}", name=f"vp{j}"
                                    )
                                    for j in range(4)
                                ]
                            psums = vpsums[rtg]
                            ht = hpool.tile([128, 4, 512], fp16, tag="ht", name="ht")
                            heng = nc.sync if ktg % 2 == 0 else nc.scalar
                            heng.dma_start(
                                out=ht,
                                in_=hkv3[
                                    :,
                                    ktg * 4 : (ktg + 1) * 4,
                                    rtg * 512 : (rtg + 1) * 512,
                                ],
                            )
                            for kl in range(4):
                                kt = ktg * 4 + kl
                                wh = vq[kt // 8]
                                for j in range(4):
                                    nc.tensor.matmul(
                                        psums[j],
                                        ht[:, kl, j * 128 : (j + 1) * 128],
                                        wh[:, kt % 8, :],
                                        start=(kt == 0),
                                        stop=(kt == KT - 1),
                                    )
                            if ktg == 7:
                                for j in range(4):
                                    if j % 2 == 0:
                                        nc.scalar.copy(
                                            out=v_s[:, rtg * 4 + j, :], in_=psums[j]
                                        )
                                    else:
                                        nc.vector.tensor_copy(
                                            out=v_s[:, rtg * 4 + j, :], in_=psums[j]
                                        )

                        vunits = [(rtg, ktg) for rtg in range(4) for ktg in range(8)]
                        vcur = [0]

                        def emit_vunits(k):
                            while k > 0 and vcur[0] < len(vunits):
                                v_unit(*vunits[vcur[0]])
                                vcur[0] += 1
                                k -= 1

                        # 11 units up front: all of rtg0 + 3 of rtg1, so the
                        # 5 fill slots of head (0,0) finish rtg1 exactly when
                        # ctx(0,0) needs v rows 0-1023.
                        emit_vunits(11)
                        for h in range(HPC):
                            ensure_bias_ahead(h)
                            bias_t = bias_tiles.pop((0, h))
                            probsT = ptpool.tile(
                                [128, 8, 512], fp16, tag="probsT", name="probsT"
                            )
                            pN = [None] * 4
                            pN[0] = scores_softmax(sps0, 0, h, 0, bias_t)
                            emit_vunits(1)
                            pN[1] = scores_softmax(sps0, 0, h, 1, bias_t)
                            emit_vunits(1)
                            transposes(pN[0], probsT, 0)
                            pN[2] = scores_softmax(sps0, 0, h, 2, bias_t)
                            emit_vunits(1)
                            transposes(pN[1], probsT, 1)
                            pN[3] = scores_softmax(sps0, 0, h, 3, bias_t)
                            emit_vunits(1)
                            transposes(pN[2], probsT, 2)
                            transposes(pN[3], probsT, 3)
                            emit_vunits(1)
                            ctx(0, h, probsT)
                        emit_vunits(len(vunits))  # the remainder (1 unit)

                    # --- blocks 1-3: outproj(n-1) fills the softmax stalls ---
                    with (
                        tc.tile_pool(name="spsum", bufs=2, space="PSUM") as sps,
                        tc.tile_pool(name="opsum", bufs=2, space="PSUM") as ops,
                    ):
                        for idx in range(HPC, len(heads)):
                            n, h = heads[idx]
                            ensure_bias_ahead(idx)
                            bias_t = bias_tiles.pop((n, h))
                            probsT = ptpool.tile(
                                [128, 8, 512], fp16, tag="probsT", name="probsT"
                            )
                            pN = [None] * 4
                            pN[0] = scores_softmax(sps, n, h, 0, bias_t)
                            pN[1] = scores_softmax(sps, n, h, 1, bias_t)
                            outproj_chunk(ops, n - 1, range(h * 8, h * 8 + 4))
                            transposes(pN[0], probsT, 0)
                            pN[2] = scores_softmax(sps, n, h, 2, bias_t)
                            transposes(pN[1], probsT, 1)
                            pN[3] = scores_softmax(sps, n, h, 3, bias_t)
                            outproj_chunk(ops, n - 1, range(h * 8 + 4, h * 8 + 6))
                            transposes(pN[2], probsT, 2)
                            transposes(pN[3], probsT, 3)
                            outproj_chunk(ops, n - 1, range(h * 8 + 6, h * 8 + 8))
                            ctx(n, h, probsT)
                        outproj_chunk(ops, NB - 1, range(KT))

    nc.compile()
    return nc


_NC_CACHE = None


def _prep_in_maps(
    hidden_q, hidden_kv, attention_mask, position_bias, Wq, Wk, Wv, Wo
):
    hqT = np.ascontiguousarray(
        np.asarray(hidden_q, dtype=np.float32).reshape(R, D).T
    ).astype(np.float16)
    hkvT = np.ascontiguousarray(
        np.asarray(hidden_kv, dtype=np.float32).reshape(R, D).T
    ).astype(np.float16)
    mask = np.asarray(attention_mask)
    pb = np.asarray(position_bias, dtype=np.float32)

    in_maps = []
    for c in range(NCORES):
        h0 = c * HPC
        bias_c = np.where(
            mask[:, None, :, :], pb[:, h0 : h0 + HPC], np.float32(MASK_NEG)
        ).astype(np.float16)
        in_maps.append(
            {
                "hqT": hqT,
                "hkvT": hkvT,
                "wq": np.ascontiguousarray(Wq[:, h0 * DH : (h0 + HPC) * DH]).astype(np.float16),
                "wk": np.ascontiguousarray(Wk[:, h0 * DH : (h0 + HPC) * DH]).astype(np.float16),
                "wv": np.ascontiguousarray(Wv[:, h0 * DH : (h0 + HPC) * DH]).astype(np.float16),
                "wo": np.ascontiguousarray(
                    Wo[h0 * DH : (h0 + HPC) * DH, :]
                ).astype(np.float16),
                "bias": bias_c,
            }
        )
    return in_maps


def kernel(
    hidden_q: np.ndarray,
    hidden_kv: np.ndarray,
    attention_mask: np.ndarray,
    position_bias: np.ndarray,
    Wq: np.ndarray,
    Wk: np.ndarray,
    Wv: np.ndarray,
    Wo: np.ndarray,
) -> np.ndarray:
    from concourse.bass_utils import run_bass_kernel_spmd

    global _NC_CACHE
    if _NC_CACHE is None:
        _NC_CACHE = _build_core_kernel()
    nc = _NC_CACHE

    in_maps = _prep_in_maps(
        hidden_q, hidden_kv, attention_mask, position_bias, Wq, Wk, Wv, Wo
    )
    res = run_bass_kernel_spmd(nc, in_maps, list(range(NCORES)))
    acc = res.results[0]["outT"].astype(np.float32)
    for c in range(1, NCORES):
        acc += res.results[c]["outT"]
    return np.ascontiguousarray(acc.T).reshape(B, S, D)
